# revision 9
# baseline (speedup 1.0000x reference)
"""BiMamba (bidirectional Mamba-1 block) Trainium2 kernel, pipelined.

Problem: B=2, L=1024, d_model=768, d_inner=1536, d_state=16, dt_rank=48,
d_conv=4; two directions (fwd on x, rev on flip(x)) sharing in/out
projections, outputs added.

Sharding over 8 NeuronCores: core = (direction, quarter of d_inner).
Cores 0-3 forward, 4-7 reverse (host flips x along L for them and unflips
their partial outputs).  Each core owns 384 d_inner channels = 3 partition
blocks of 128.

On-device layout: channels on partitions, time on the free dimension
(t = b*1024 + l, batches concatenated), chunks of TC=512.  The program is
software-pipelined per chunk with 1-chunk lookahead:

    A(c0) A(c1) B(c0) A(c2) B(c1) A(c3) B(c2) B(c3)

where A = in_proj u+z, conv, silus, x_proj partial, bf16 AllReduce,
dtT/B/C fan-out DMAs, and B = dt_proj, softplus, dA, dBu, scan, y*C,
state-sum + skip on PE, gate, out_proj.  Since every engine executes its
stream in order, interleaving the emissions is what overlaps phase-A
matmuls of chunk k+1 with phase-B elementwise work of chunk k; in the
two-phase baseline DVE+Pool idled for the first ~45us while PE drained
all four chunks of phase A.  Chunk order (b0,t0),(b1,t0),(b0,t1),(b1,t1)
keeps the carry dependency two slots apart.

ACT table discipline: each chunk's A emits its six Silu-LUT ops in one
burst, then an anchored LoadActFuncSet pins the combined Exp+Ln table
(natural_log_exp_and_others) for the following B burst (Exp x3 softplus,
Ln x3, Exp x48 dA); Copy-func ops run under any table.  2 flips per
chunk, ~9 table loads total.

Engine split per chunk (TimelineSim cost model):
  - PE  (~34 us/chunk): all matmuls; y's sum over d_state runs as 16
    identity matmuls accumulating in PSUM with the u*D skip as a 17th.
  - ACT (~42): dA = exp(A_s * delta) via per-partition scale APs, silus,
    softplus, PSUM->SBUF copies.
  - DVE (~42): all 48 scans (only DVE has tensor_tensor_scan), dBu sg0
    (8-wide bf16 TT at the 2x DVE rate), y*C sg1 (8-wide), w, gate,
    carries.
  - Pool (~42): dBu sg1 + y*C sg0 as 8-wide gpsimd scalar_tensor_tensor
    ops: (a mult 1.0) mult b.  STT lowers to InstTensorScalarPtr whose
    gpsimd efficiency is the 0.60 default instead of TensorTensor's 0.42
    "Multiply" entry -- 1.42x faster for the same elementwise multiply.

The AllReduce runs in bf16 so the reduced dt rows DMA straight into the
dt^T tile and the B/C rows broadcast to 128 partitions directly from the
collective output (DMA cannot cast).  The collective is emitted inside
A(ck), before any of B(ck-1)'s Pool ops enter the gpsimd queue, and the
dtT/B/C fan-out DMAs are emitted right after it so no DMA ring order
inversion can stall them behind chunk k+1's cin.  Output partials are
stored bf16 and summed f32 on the host.
"""
import sys

sys.path.insert(0, "/opt/trn_rl_repo")

import numpy as np
import ml_dtypes

import concourse.bass as bass
import concourse.bacc as bacc
import concourse.mybir as mybir
import concourse.tile as tile
from concourse.bass_utils import run_bass_kernel_spmd
from concourse.hw_specs import get_activation_tables

# ---- problem constants ----
B, L, DM, E, S, R, K = 2, 1024, 768, 1536, 16, 48, 4
NCORES = 8
Q = 4                      # d_inner quarters per direction
DSL = E // Q               # 384 channels per core
NDB = DSL // 128           # 3 partition blocks
T = B * L                  # 2048 tokens, b-major
TC = 512                   # time chunk
NTC = L // TC              # chunks per batch
SG = 2                     # d_state groups
SPG = S // SG              # 8 states per group
NR = R + 2 * S             # 80
NKC = DM // 128            # 6 k-chunks of d_model

bf16 = mybir.dt.bfloat16
f32 = mybir.dt.float32
STAG = (0.0, 0.036, 0.074, 0.112)   # per-chunk silu release times (ms)
MUL = mybir.AluOpType.mult
ADD = mybir.AluOpType.add
AF = mybir.ActivationFunctionType

_NC_CACHE = None


def _bcast_free(ap_row, n):
    """Broadcast an AP of shape [p, F] to [p, n, F] with step-0 middle dim."""
    return bass.AP(tensor=ap_row.tensor, offset=ap_row.offset,
                   ap=[ap_row.ap[0], [0, n], ap_row.ap[1]])


def build_nc(sim_mode=False):
    """sim_mode=True: single-core, collective replaced by a DRAM->DRAM DMA
    (same dataflow deps) so TimelineSim can run the kernel."""
    nc = bacc.Bacc("TRN2", target_bir_lowering=False, debug=False,
                   num_devices=1 if sim_mode else NCORES)

    # ---------------- I/O ----------------
    xT = nc.dram_tensor("xT", [DM, T], bf16, kind="ExternalInput")
    wu = nc.dram_tensor("wu", [128, NKC, DSL], bf16, kind="ExternalInput")
    wz = nc.dram_tensor("wz", [128, NKC, DSL], bf16, kind="ExternalInput")
    wcd = nc.dram_tensor("wcd", [128, NDB, K, 128], bf16, kind="ExternalInput")
    wxp = nc.dram_tensor("wxp", [128, NDB, NR], bf16, kind="ExternalInput")
    wdt = nc.dram_tensor("wdt", [R, DSL], bf16, kind="ExternalInput")
    wo = nc.dram_tensor("wo", [128, NDB, DM], bf16, kind="ExternalInput")
    dtb = nc.dram_tensor("dtb", [128, NDB, 1], f32, kind="ExternalInput")
    Aneg = nc.dram_tensor("Aneg", [128, NDB, S], f32, kind="ExternalInput")
    cb = nc.dram_tensor("cb", [128, NDB, 1], f32, kind="ExternalInput")
    wdp = nc.dram_tensor("wdp", [128, NDB, 128], bf16, kind="ExternalInput")
    ident = nc.dram_tensor("ident", [128, 128], bf16, kind="ExternalInput")
    opart = nc.dram_tensor("opart", [DM, T], bf16, kind="ExternalOutput")

    nle = None  # act table id, resolved once below

    with tile.TileContext(nc) as tc:
        import contextlib
        ctx = contextlib.ExitStack()
        with ctx:
            ctx.enter_context(nc.allow_low_precision(reason="deliberate bf16 pipeline"))
            singles = ctx.enter_context(tc.tile_pool(name="singles", bufs=1))
            xpool = ctx.enter_context(tc.tile_pool(name="xs", bufs=2))
            big = ctx.enter_context(tc.tile_pool(name="big", bufs=2))
            hpool = ctx.enter_context(tc.tile_pool(name="hp", bufs=3))
            bcpool = ctx.enter_context(tc.tile_pool(name="bc", bufs=2))
            small = ctx.enter_context(tc.tile_pool(name="small", bufs=3))
            actp = ctx.enter_context(tc.tile_pool(name="actp", bufs=3))
            redp = ctx.enter_context(tc.tile_pool(name="redp", bufs=2))
            y2pool = ctx.enter_context(tc.tile_pool(name="y2", bufs=4))
            otpool = ctx.enter_context(tc.tile_pool(name="otp", bufs=2))
            psum = ctx.enter_context(tc.tile_pool(name="psum", bufs=4, space="PSUM"))
            psdt = ctx.enter_context(tc.tile_pool(name="psdt", bufs=1, space="PSUM"))
            psumy = ctx.enter_context(tc.tile_pool(name="psumy", bufs=1, space="PSUM"))
            psumo = ctx.enter_context(tc.tile_pool(name="psumo", bufs=2, space="PSUM"))
            dram = ctx.enter_context(tc.tile_pool(name="dram", bufs=1, space="DRAM"))

            # ------------- persistent weights -------------
            # Phase-A weights first so A(c0)'s matmuls aren't queued behind
            # B-only weights on the DMA rings.
            wu_sb = singles.tile([128, NKC, DSL], bf16)   # [k, kc, m]
            nc.sync.dma_start(out=wu_sb, in_=wu.ap())
            wz_sb = singles.tile([128, NKC, DSL], bf16)
            nc.sync.dma_start(out=wz_sb, in_=wz.ap())
            wcd_sb = singles.tile([128, NDB, K, 128], bf16)
            nc.sync.dma_start(out=wcd_sb, in_=wcd.ap())
            wxp_sb = singles.tile([128, NDB, NR], bf16)
            nc.sync.dma_start(out=wxp_sb, in_=wxp.ap())
            cb_sb = singles.tile([128, NDB, 1], f32)
            nc.sync.dma_start(out=cb_sb, in_=cb.ap())
            # B-phase weights are DMA'd after A(c0) is emitted (see below) so
            # the first chunk's xs/compute isn't queued behind them.
            wdt_sb = singles.tile([R, DSL], bf16)
            wo_sb = singles.tile([128, NDB, DM], bf16)
            dtb_sb = singles.tile([128, NDB, 1], f32)
            A_sb = singles.tile([128, NDB, S], f32)
            id_sb = singles.tile([128, 128], bf16)
            wdp_sb = singles.tile([128, NDB, 128], bf16)

            def emit_b_weight_dmas():
                nc.sync.dma_start(out=wdt_sb, in_=wdt.ap())
                nc.sync.dma_start(out=wo_sb, in_=wo.ap())
                nc.sync.dma_start(out=dtb_sb, in_=dtb.ap())
                nc.sync.dma_start(out=A_sb, in_=Aneg.ap())
                nc.sync.dma_start(out=id_sb, in_=ident.ap())
                nc.sync.dma_start(out=wdp_sb, in_=wdp.ap())

            # persistent activations
            u_sb = singles.tile([128, NDB, B, 3 + L], bf16)  # conv input, 3-pad
            uc_sb = singles.tile([128, NDB, T], bf16)        # silu(conv(u))
            sz_sb = singles.tile([128, NDB, T], bf16)        # silu(z)
            dtT_sb = singles.tile([R, T], bf16)              # reduced dt^T
            carry = [[singles.tile([128, SPG], f32, tag=f"carry{db}_{sg}",
                                   name=f"carry{db}_{sg}")
                      for sg in range(SG)] for db in range(NDB)]

            for db in range(NDB):
                for b in range(B):
                    nc.vector.memset(u_sb[:, db, b, 0:3], 0.0)

            nle = list(get_activation_tables(nc.m.arch)).index(
                "natural_log_exp_and_others")
            couts = {}
            bcs = {}

            # Chunk order: batches interleaved so the scan-carry dependency
            # (b, tcn=0) -> (b, tcn=1) is two pipeline slots apart.
            chunks = [(b, tcn) for tcn in range(NTC) for b in range(B)]

            def emit_a(ci):
                b, tcn = chunks[ci]
                t0 = b * L + tcn * TC
                xs = xpool.tile([128, NKC, TC], bf16, tag="xs")
                xin = xT.ap()[0:128, t0:t0 + TC]
                nc.sync.dma_start(
                    out=xs,
                    in_=bass.AP(tensor=xin.tensor, offset=xin.offset,
                                ap=[xin.ap[0], [128 * T, NKC], xin.ap[1]]))
                for db in range(NDB):
                    pu = psum.tile([128, TC], f32, tag="mm")
                    for kc in range(NKC):
                        nc.tensor.matmul(
                            pu, wu_sb[:, kc, db * 128:(db + 1) * 128],
                            xs[:, kc, :],
                            start=(kc == 0), stop=(kc == NKC - 1))
                    nc.scalar.copy(
                        u_sb[:, db, b, 3 + tcn * TC: 3 + (tcn + 1) * TC], pu)
                    pc = psum.tile([128, TC], f32, tag="mm")
                    for k in range(K):
                        nc.tensor.matmul(
                            pc, wcd_sb[:, db, k, :],
                            u_sb[:, db, b, tcn * TC + k: tcn * TC + k + TC],
                            start=(k == 0), stop=(k == K - 1))
                    # Stagger later chunks' silus to roughly when the prior
                    # chunk's Exp/Ln/dA burst drains so each chunk's six
                    # Silu-LUT ops batch into one table residency instead of
                    # interleaving reloads into the B bursts.
                    with tc.tile_wait_until(STAG[ci], enable=ci >= 1):
                        nc.scalar.activation(uc_sb[:, db, t0:t0 + TC], pc,
                                             AF.Silu,
                                             bias=cb_sb[:, db, :],
                                             scale=1.0)
                    pz = psum.tile([128, TC], f32, tag="mm")
                    for kc in range(NKC):
                        nc.tensor.matmul(
                            pz, wz_sb[:, kc, db * 128:(db + 1) * 128],
                            xs[:, kc, :],
                            start=(kc == 0), stop=(kc == NKC - 1))
                    with tc.tile_wait_until(STAG[ci], enable=ci >= 1):
                        nc.scalar.activation(sz_sb[:, db, t0:t0 + TC], pz,
                                             AF.Silu, scale=1.0)
                px = psum.tile([NR, TC], f32, tag="mm")
                for db in range(NDB):
                    nc.tensor.matmul(px, wxp_sb[:, db, :], uc_sb[:, db, t0:t0 + TC],
                                     start=(db == 0), stop=(db == NDB - 1))
                # bf16 AllReduce: dtT and the B/C broadcasts then read
                # cout directly (DMA cannot cast).
                xps = redp.tile([NR, TC], bf16, tag="xps")
                nc.scalar.copy(xps, px)
                cin = dram.tile([NR, TC], bf16, tag=f"cin{ci}",
                                name=f"cin{ci}")
                cout = dram.tile([NR, TC], bf16, tag=f"cout{ci}",
                                 name=f"cout{ci}")
                nc.sync.dma_start(out=cin, in_=xps)
                if sim_mode:
                    nc.sync.dma_start(out=cout, in_=cin)
                else:
                    nc.gpsimd.collective_compute(
                        "AllReduce", ADD,
                        replica_groups=[[0, 1, 2, 3], [4, 5, 6, 7]],
                        ins=[cin.opt()], outs=[cout.opt()],
                    )
                couts[ci] = cout
                nc.sync.dma_start(out=dtT_sb[:, t0:t0 + TC],
                                  in_=cout[0:R, :])
                # B/C rows broadcast to all 128 partitions straight from the
                # collective output: 1 descriptor-batched DMA per (sg, B/C).
                # Emitted here (not in B) so they precede chunk k+1's cin on
                # the DMA rings.
                bbc = []
                cbc = []
                for sg in range(SG):
                    bt = bcpool.tile([128, SPG, TC], bf16, tag="bbc")
                    ct = bcpool.tile([128, SPG, TC], bf16, tag="cbc")
                    rb = cout[R + sg * SPG: R + sg * SPG + 1, :]
                    nc.sync.dma_start(
                        out=bt,
                        in_=bass.AP(tensor=rb.tensor, offset=rb.offset,
                                    ap=[[0, 128], [TC, SPG], rb.ap[-1]]))
                    rc = cout[R + S + sg * SPG: R + S + sg * SPG + 1, :]
                    nc.sync.dma_start(
                        out=ct,
                        in_=bass.AP(tensor=rc.tensor, offset=rc.offset,
                                    ap=[[0, 128], [TC, SPG], rc.ap[-1]]))
                    bbc.append(bt)
                    cbc.append(ct)
                bcs[ci] = (bbc, cbc)

            def emit_b(ci):
                b, tcn = chunks[ci]
                t0 = b * L + tcn * TC
                bbc, cbc = bcs[ci]

                # Anchored act-table pin: becomes ready with this chunk's
                # reduced dtT, i.e. right before the chunk's Exp/Ln burst in
                # the scheduled ACT order, and its program index precedes the
                # burst.  Keeps the post-schedule load-insertion pass from
                # alternating the single-function exp/ln tables; Silu bursts
                # of later A-chunks sort after this chunk's burst by index.
                ld = mybir.InstLoadActFuncSet(
                    name=nc.get_next_instruction_name(),
                    ins=[nc.scalar.lower_ap(dtT_sb[0:1, t0:t0 + 1])],
                    outs=[], act_func_set_id=nle)
                nc.scalar.add_instruction(ld)

                # dt_proj -> softplus for all three blocks first: the
                # in-order ACT engine then runs Exp x3, Ln x3, Exp x48 in
                # one Exp+Ln table residency.
                deltas = []
                e1s = []
                for db in range(NDB):
                    pdt = psdt.tile([128, TC], f32, tag="pdt")
                    nc.tensor.matmul(pdt, wdt_sb[:, db * 128:(db + 1) * 128],
                                     dtT_sb[:, t0:t0 + TC], start=True, stop=True)
                    e1 = actp.tile([128, TC], bf16, tag="e1")
                    nc.scalar.activation(e1, pdt, AF.Exp,
                                         bias=dtb_sb[:, db, :], scale=1.0)
                    e1s.append(e1)
                for db in range(NDB):
                    delta = small.tile([128, TC], bf16, tag="delta")
                    nc.scalar.activation(delta, e1s[db], AF.Ln,
                                         bias=1.0, scale=1.0)
                    deltas.append(delta)
                e1s = None

                ys = []
                for db in range(NDB):
                    delta = deltas[db]
                    # w = delta * uc
                    w = small.tile([128, TC], bf16, tag="w")
                    nc.vector.tensor_tensor(w, delta, uc_sb[:, db, t0:t0 + TC], MUL)

                    py = psumy.tile([128, TC], f32, tag="py")
                    for sg in range(SG):
                        s0 = sg * SPG
                        da = big.tile([128, SPG, TC], bf16, tag="da")
                        for i in range(SPG):
                            nc.scalar.activation(da[:, i, :], delta, AF.Exp,
                                                 scale=A_sb[:, db, s0 + i:s0 + i + 1])
                        dbu = big.tile([128, SPG, TC], bf16, tag="dbu")
                        if sg == 0:
                            nc.vector.tensor_tensor(dbu, _bcast_free(w, SPG),
                                                    bbc[sg], MUL)
                        else:
                            # Pool takes sg1's dBu as two 4-wide TTs (the
                            # only walrus-legal Q7 elementwise shape)
                            for q0 in range(0, SPG, 4):
                                nc.gpsimd.tensor_tensor(
                                    dbu[:, q0:q0 + 4, :],
                                    _bcast_free(w, 4),
                                    bbc[sg][:, q0:q0 + 4, :], MUL)
                        h = hpool.tile([128, SPG, TC], bf16, tag="h")
                        for i in range(SPG):
                            init = 0.0 if tcn == 0 else carry[db][sg][:, i:i + 1]
                            nc.vector.tensor_tensor_scan(
                                h[:, i, :], da[:, i, :], dbu[:, i, :], init,
                                MUL, ADD)
                        if tcn + 1 < NTC:
                            nc.vector.tensor_copy(carry[db][sg], h[:, :, TC - 1])
                        yp = big.tile([128, SPG, TC], bf16, tag="yp")
                        if sg == 0:
                            # Pool: s0..5 (4+2 wide), DVE: s6..7
                            nc.gpsimd.tensor_tensor(
                                yp[:, 0:4, :], h[:, 0:4, :],
                                cbc[sg][:, 0:4, :], MUL)
                            nc.gpsimd.tensor_tensor(
                                yp[:, 4:6, :], h[:, 4:6, :],
                                cbc[sg][:, 4:6, :], MUL)
                            nc.vector.tensor_tensor(
                                yp[:, 6:8, :], h[:, 6:8, :],
                                cbc[sg][:, 6:8, :], MUL)
                        else:
                            nc.vector.tensor_tensor(yp, h, cbc[sg], MUL)
                        for i in range(SPG):
                            nc.tensor.matmul(py, id_sb, yp[:, i, :],
                                             start=(sg == 0 and i == 0),
                                             stop=False)
                    # skip connection via PE: py += diag(Dp) @ uc,
                    # then gate straight from PSUM: y2 = py * silu(z)
                    nc.tensor.matmul(py, wdp_sb[:, db, :],
                                     uc_sb[:, db, t0:t0 + TC],
                                     start=False, stop=True)
                    y2t = y2pool.tile([128, TC], bf16, tag="y2")
                    nc.vector.tensor_tensor(y2t, py, sz_sb[:, db, t0:t0 + TC], MUL)
                    ys.append(y2t)

                # out_proj partial for this chunk
                for mc in range(NKC):
                    po = psumo.tile([128, TC], f32, tag="po")
                    for db in range(NDB):
                        nc.tensor.matmul(
                            po, wo_sb[:, db, mc * 128:(mc + 1) * 128], ys[db],
                            start=(db == 0), stop=(db == NDB - 1))
                    ot = otpool.tile([128, TC], bf16, tag="ot")
                    nc.scalar.copy(ot, po)
                    nc.sync.dma_start(
                        out=opart.ap()[mc * 128:(mc + 1) * 128, t0:t0 + TC],
                        in_=ot)

            # Software pipeline, 1-chunk lookahead.
            NCH = len(chunks)
            emit_a(0)
            emit_b_weight_dmas()
            for ci in range(NCH):
                if ci + 1 < NCH:
                    emit_a(ci + 1)
                emit_b(ci)
    nc.compile()
    return nc


def _get_nc():
    global _NC_CACHE
    if _NC_CACHE is None:
        _NC_CACHE = build_nc()
    return _NC_CACHE


def _bf(a):
    return np.ascontiguousarray(a).astype(ml_dtypes.bfloat16)


def kernel(**inputs):
    hs = np.asarray(inputs["hidden_states"], dtype=np.float32)  # (B, L, DM)
    in_w = np.asarray(inputs["in_proj_w"], dtype=np.float32)    # (2E, DM)
    out_w = np.asarray(inputs["out_proj_w"], dtype=np.float32)  # (DM, E)
    ident = np.eye(128, dtype=np.float32)

    in_maps = []
    for c in range(NCORES):
        d = "f" if c < 4 else "r"
        q = c % 4
        sl = slice(q * DSL, (q + 1) * DSL)
        x = hs if d == "f" else hs[:, ::-1, :]
        xTh = np.ascontiguousarray(x.transpose(2, 0, 1).reshape(DM, T))

        cw = np.asarray(inputs[f"conv_w_{d}"], dtype=np.float32)[sl]   # (384, 4)
        Dq = np.asarray(inputs[f"D_{d}"], np.float32)[sl]
        wdph = np.zeros((NDB, 128, 128), np.float32)
        for db in range(NDB):
            np.fill_diagonal(wdph[db], Dq[db * 128:(db + 1) * 128])
        wcdh = np.zeros((NDB, K, 128, 128), np.float32)
        for db in range(NDB):
            for k in range(K):
                np.fill_diagonal(wcdh[db, k], cw[db * 128:(db + 1) * 128, k])

        xpw = np.asarray(inputs[f"x_proj_w_{d}"], dtype=np.float32)    # (80, E)
        wxph = np.stack([xpw[:, q * DSL + db * 128: q * DSL + (db + 1) * 128].T
                         for db in range(NDB)])                         # (3,128,80)
        dtw = np.asarray(inputs[f"dt_w_{d}"], dtype=np.float32)[sl]    # (384, 48)
        woh = np.stack([out_w[:, q * DSL + db * 128: q * DSL + (db + 1) * 128].T
                        for db in range(NDB)])                          # (3,128,768)

        def p_major(a):
            # (NDB, 128, ...) -> (128, NDB, ...) contiguous
            return np.ascontiguousarray(np.moveaxis(a, 1, 0))

        in_maps.append({
            "xT": _bf(xTh),
            "wu": _bf(p_major(in_w[sl].T.reshape(NKC, 128, DSL))),
            "wz": _bf(p_major(in_w[E:][sl].T.reshape(NKC, 128, DSL))),
            "wcd": _bf(np.ascontiguousarray(wcdh.transpose(2, 0, 1, 3))),
            "wxp": _bf(p_major(wxph)),
            "wdt": _bf(dtw.T),
            "wo": _bf(p_major(woh)),
            "dtb": p_major(
                np.asarray(inputs[f"dt_b_{d}"], np.float32)[sl].reshape(NDB, 128, 1)),
            "Aneg": p_major(
                (-np.exp(np.asarray(inputs[f"A_log_{d}"], np.float32)[sl]))
                .reshape(NDB, 128, S)),
            "cb": p_major(
                np.asarray(inputs[f"conv_b_{d}"], np.float32)[sl].reshape(NDB, 128, 1)),
            "wdp": _bf(p_major(wdph)),
            "ident": _bf(ident),
        })

    nc = _get_nc()
    res = run_bass_kernel_spmd(nc, in_maps, core_ids=list(range(NCORES)))

    acc_f = np.zeros((DM, T), np.float32)
    acc_r = np.zeros((DM, T), np.float32)
    for c in range(NCORES):
        if c < 4:
            acc_f += res.results[c]["opart"].astype(np.float32)
        else:
            acc_r += res.results[c]["opart"].astype(np.float32)
    out_f = acc_f.reshape(DM, B, L).transpose(1, 2, 0)
    out_r = acc_r.reshape(DM, B, L).transpose(1, 2, 0)[:, ::-1, :]
    return np.ascontiguousarray(out_f + out_r, dtype=np.float32)
